# revision 6
# baseline (speedup 1.0000x reference)
# Multi-head causal self-attention (B=2, S=2048, D=768, H=12) on 8 NeuronCores.
#
# Sharding: (batch, head-group) across cores. Core c handles batch c//4 and
# heads 3*(c%4) .. 3*(c%4)+2. Each core computes its heads' Q/K/V projections
# (column-sharded), the causal attention for those heads, and a row-sharded
# partial of the output projection. Host sums the 4 partials per batch + bo.
#
# Engine plan (v3):
#  - PE alternates between two tile-size phases per query chunk so the array
#    never mode-switch-drains mid-stream:
#      (128,128) phase: QK/V projections, AV (M=65, ones column gives the
#                       softmax denominator) of the previous chunk.
#      (64,128) phase: scores -- heads 0,1 are PAIR-STACKED on partitions
#                       (qT/klo [128] = h0 on 0-63, h1 on 64-127) and issued
#                       to row tiles T0 and T8 which execute concurrently;
#                       head 2 runs on T0.  Output projection (K=64) of the
#                       previous chunk is sprinkled in as filler.
#  - ACT: only exp (one table set, no Ln).
#  - DVE: PSUM evacuations (fused QK bias add), 1/Z reciprocal, ctx normalize.
#  - GPSIMD: causal-mask multiplies on exp tiles and 1/Z partition-broadcast.
#  - Output DMA'd in bf16; host accumulates partials in fp32.

import sys

import ml_dtypes
import numpy as np

sys.path.insert(0, "/opt/trn_rl_repo")

import concourse.bass as bass  # noqa: E402
import concourse.mybir as mybir  # noqa: E402
import concourse.tile as tile  # noqa: E402
from concourse.bass import ts  # noqa: E402
from concourse.bass_utils import run_bass_kernel_spmd  # noqa: E402

F32 = mybir.dt.float32
BF16 = mybir.dt.bfloat16
AF = mybir.ActivationFunctionType
MUL = mybir.AluOpType.mult
ADD = mybir.AluOpType.add
NPBF16 = ml_dtypes.bfloat16

B, S, D, H, HD = 2, 2048, 768, 12, 64
HPC = 3               # heads per core
DQK = 2 * HPC * HD    # 384
DV = HPC * HD         # 192
P = 128
IC = S // 512         # 4 query chunks of 512
KC = D // P           # 6 contraction chunks
NIO = S // P          # 16 token chunks of 128


def _split_excess_waits(nc, max_waits=1):
    # walrus in this env rejects instructions carrying more than ~1-2
    # sync-waits. Move excess waits onto preceding same-engine nops
    # (sequencer executes the nop's wait, then the instruction's).
    n_split = 0
    for func in nc.m.functions:
        for blk in func.blocks:
            insts = blk.instructions
            out = []
            changed = False
            for inst in insts:
                si = inst.sync_info
                waits = list(si.on_wait) if si and si.on_wait else []
                if len(waits) > max_waits:
                    changed = True
                    for j, w in enumerate(waits[:-max_waits]):
                        out.append(
                            mybir.InstNoOp(
                                name=f"{inst.name}-wsplit{j}",
                                engine=inst.engine,
                                ins=[],
                                outs=[],
                                sync_info=mybir.SyncInfo(
                                    on_wait=[w], on_update=[]
                                ),
                            )
                        )
                        n_split += 1
                    inst.sync_info = mybir.SyncInfo(
                        on_wait=waits[-max_waits:],
                        on_update=list(si.on_update) if si.on_update else [],
                    )
                out.append(inst)
            if changed:
                blk.instructions = out
    return n_split


def _build_module():
    nc = bass.Bass()
    xt_d = nc.dram_tensor("xt", [D, S], BF16, kind="ExternalInput")
    wqk_d = nc.dram_tensor("wqk", [D, DQK], BF16, kind="ExternalInput")
    bqk_d = nc.dram_tensor("bqk", [P, HPC], F32, kind="ExternalInput")
    wv_d = nc.dram_tensor("wv", [D, DV], BF16, kind="ExternalInput")
    wos_d = nc.dram_tensor("wos", [HD, HPC, D], BF16, kind="ExternalInput")
    mask_d = nc.dram_tensor("mask", [P, 4, 2, 512], BF16, kind="ExternalInput")
    out_d = nc.dram_tensor("out", [S, D], BF16, kind="ExternalOutput")
    scratch_d = nc.dram_tensor("scratch", [HD + 1, 512], F32)

    with tile.TileContext(nc) as tc:
        with (
            tc.tile_pool(name="const", bufs=1) as cp,
            tc.tile_pool(name="exp", bufs=26) as exp_p,
            tc.tile_pool(name="zr", bufs=4) as zr_p,
            tc.tile_pool(name="zd", bufs=3, space="DRAM") as zd_p,
            tc.tile_pool(name="outp", bufs=2) as op,
            tc.tile_pool(name="proj", bufs=2, space="PSUM") as proj_p,
            tc.tile_pool(name="scps", bufs=2, space="PSUM") as sc_p,
            tc.tile_pool(name="avps", bufs=2, space="PSUM") as av_p,
        ):
            # ---- resident SBUF tensors ----
            # V with a ones column (col HD) for the softmax denominator; the
            # memset also provides an all-ones PE warm-up operand.
            v1 = cp.tile([P, NIO, HPC, HD + 1], BF16)
            nc.gpsimd.memset(v1, 1.0)

            wqk_sb = cp.tile([P, KC, DQK], BF16)
            nc.sync.dma_start(wqk_sb, wqk_d.rearrange("(kc p) d -> p kc d", p=P))
            bqk_sb = cp.tile([P, HPC], F32)
            nc.sync.dma_start(bqk_sb, bqk_d[:])
            wv_sb = cp.tile([P, KC, DV], BF16)
            nc.sync.dma_start(wv_sb, wv_d.rearrange("(kc p) d -> p kc d", p=P))

            # whole transposed input, resident: [128, kc, 2048]
            xt_sb = cp.tile([P, KC, S], BF16)
            xt_r = xt_d.rearrange("(kc p) t -> p kc t", p=P)
            for kc in range(KC):
                for ic in range(IC):
                    nc.sync.dma_start(
                        xt_sb[:, kc, ts(ic, 512)], xt_r[:, kc, ts(ic, 512)]
                    )

            wos_sb = cp.tile([HD, HPC, D], BF16)
            nc.sync.dma_start(wos_sb, wos_d[:])
            mask_sb = cp.tile([P, 4, 2, 512], BF16)
            nc.sync.dma_start(mask_sb, mask_d[:])

            # pair-stacked Q^T/K^T for heads 0,1; separate tiles for head 2
            qTp = cp.tile([P, S], BF16)
            klp = cp.tile([P, S], BF16)
            qT2 = cp.tile([HD, S], BF16)
            kl2 = cp.tile([HD, S], BF16)
            ctxT = cp.tile([HD, HPC, S], BF16)    # normalized ctx^T [d, h, i]

            # ---- PE warm-up: (128,128)-mode matmuls on the ones tile ----
            warm_ps = av_p.tile([P, 512], F32, tag="av", name="warm")
            for w in range(12):
                nc.tensor.matmul(
                    warm_ps[0 : HD + 1, :],
                    lhsT=v1[:, 0, 0, :],
                    rhs=v1[:, 0:8, 0, 0:64],
                    start=True,
                    stop=(w == 11),
                )
            warm_sb = zr_p.tile([HD + 1, 512], F32, tag="zr", name="warmsb")
            nc.vector.tensor_copy(warm_sb, warm_ps[0 : HD + 1, :])
            nc.sync.dma_start(scratch_d[:], warm_sb)

            # oproj filler state: pending closures each emitting one io-unit
            pend = []

            def drain_filler(n=1):
                for _ in range(n):
                    if pend:
                        pend.pop(0)()

            def emit_oproj(ic):
                for io4 in range(4):
                    io = ic * 4 + io4

                    def unit(io=io):
                        o_sb = op.tile([P, D], BF16, tag="osb")
                        for ot, ow in ((0, 512), (1, 256)):
                            ps = proj_p.tile([P, 512], F32, tag="proj")
                            pso = ps[:, :ow]
                            for h in range(HPC):
                                nc.tensor.matmul(
                                    pso,
                                    lhsT=ctxT[:, h, ts(io, P)],
                                    rhs=wos_sb[:, h, ot * 512 : ot * 512 + ow],
                                    start=(h == 0),
                                    stop=(h == HPC - 1),
                                )
                            nc.vector.tensor_copy(
                                o_sb[:, ot * 512 : ot * 512 + ow], pso
                            )
                        nc.sync.dma_start(out_d[ts(io, P), :], o_sb)

                    pend.append(unit)

            # attention state carried between phases: ex tiles of chunk ic
            # (consumed by the AV batch inside the next (128,128) phase)
            carry = {}

            def emit_qkv_proj(ic):
                isl = ts(ic, 512)
                # QK projection, pair-packed slices:
                #  slice 0 -> [Q_h0 | Q_h1], slice 1 -> [K_h0 | K_h1],
                #  slice 2 -> [Q_h2 | K_h2]
                dsts = [(qTp[:, isl], slice(0, P)), (klp[:, isl], slice(0, P)),
                        (None, None)]
                for sl in range(HPC):
                    ps = proj_p.tile([P, 512], F32, tag="proj")
                    for kc in range(KC):
                        nc.tensor.matmul(
                            ps,
                            lhsT=wqk_sb[:, kc, ts(sl, P)],
                            rhs=xt_sb[:, kc, isl],
                            start=(kc == 0),
                            stop=(kc == KC - 1),
                        )
                    if sl < 2:
                        dst = (qTp if sl == 0 else klp)[:, isl]
                        nc.vector.tensor_scalar(
                            dst, ps, bqk_sb[:, sl : sl + 1], None, ADD,
                        )
                    else:
                        nc.vector.tensor_scalar(
                            qT2[:, isl], ps[0:HD, :],
                            bqk_sb[0:HD, sl : sl + 1], None, ADD,
                        )
                        nc.vector.tensor_scalar(
                            kl2[:, isl], ps[HD:P, :],
                            bqk_sb[HD:P, sl : sl + 1], None, ADD,
                        )

            def emit_v_proj(ic):
                for io4 in range(4):
                    io = ic * 4 + io4
                    ps = proj_p.tile([P, 512], F32, tag="proj")
                    psv = ps[:, :DV]
                    c0 = ic * 512 + io4 * P
                    for kc in range(KC):
                        nc.tensor.matmul(
                            psv,
                            lhsT=xt_sb[:, kc, c0 : c0 + P],
                            rhs=wv_sb[:, kc, :],
                            start=(kc == 0),
                            stop=(kc == KC - 1),
                        )
                    nc.vector.tensor_copy(
                        v1[:, io, :, 0:HD],
                        psv.rearrange("p (h e) -> p h e", e=HD),
                    )

            def emit_scores(ic):
                # (64,128) phase: pair scores per key chunk (T0||T8), head-2
                # scores per chunk pair (T0); exp on ACT, mask on gpsimd.
                n_j = 4 * ic + 4
                exs, exs2 = [], []

                def trim_of(jc):
                    koff = jc - 4 * ic
                    return P * koff if koff > 0 else 0

                nslots = n_j + n_j // 2
                slot = 0
                fill_at = {
                    (i + 1) * nslots // 4 for i in range(4)
                } if pend else set()

                def tick():
                    nonlocal slot
                    slot += 1
                    if slot in fill_at:
                        drain_filler(1)

                for jb in range(0, n_j, 2):
                    for jc in (jb, jb + 1):
                        # heads 0,1 concurrently on row tiles T0/T8
                        t = trim_of(jc)
                        koff = jc - 4 * ic
                        sc = sc_p.tile([P, 2, 512], F32, tag="sc",
                                       name=f"sc{ic}_{jc}")
                        for h in range(2):
                            hsl = ts(h, HD)
                            nc.tensor.matmul(
                                sc[:, h, t:],
                                lhsT=(klp[hsl, ts(jc, P)]),
                                rhs=(qTp[hsl, ic * 512 + t : (ic + 1) * 512]),
                                start=True,
                                stop=True,
                            )
                        ex = exp_p.tile([P, 2, 512], BF16, tag="ex",
                                        name=f"ex{ic}_{jc}")
                        nc.scalar.activation(ex[:, :, t:], sc[:, :, t:], AF.Exp)
                        if koff >= 0:
                            nc.gpsimd.tensor_tensor(
                                ex[:, :, t:], ex[:, :, t:],
                                mask_sb[:, koff, :, t:], MUL,
                            )
                        exs.append(ex)
                        tick()
                    # head 2, chunks jb and jb+1 in one tile
                    sc = sc_p.tile([P, 2, 512], F32, tag="sc",
                                   name=f"sc2_{ic}_{jb}")
                    for k in range(2):
                        jc = jb + k
                        t = trim_of(jc)
                        nc.tensor.matmul(
                            sc[:, k, t:],
                            lhsT=kl2[:, ts(jc, P)],
                            rhs=qT2[:, ic * 512 + t : (ic + 1) * 512],
                            start=True,
                            stop=True,
                        )
                    ex = exp_p.tile([P, 2, 512], BF16, tag="ex",
                                    name=f"ex2_{ic}_{jb}")
                    koff = jb - 4 * ic
                    if koff >= 0 and trim_of(jb + 1) > 0:
                        for k in range(2):
                            t = trim_of(jb + k)
                            nc.scalar.activation(
                                ex[:, k, t:], sc[:, k, t:], AF.Exp
                            )
                            nc.gpsimd.tensor_tensor(
                                ex[:, k, t:], ex[:, k, t:],
                                mask_sb[:, koff + k, 0, t:], MUL,
                            )
                    else:
                        nc.scalar.activation(ex, sc, AF.Exp)
                        if koff >= 0:
                            nc.gpsimd.tensor_tensor(
                                ex, ex, mask_sb[:, koff : koff + 2, 0, :], MUL,
                            )
                    exs2.append(ex)
                    tick()
                carry[ic] = (exs, exs2)

            def emit_av(ic):
                # (128,128) phase: AV for all heads of chunk ic; M=65 (ones
                # column accumulates Z); heads sequential so the two av psum
                # buffers ping-pong with the DVE norm drain.
                n_j = 4 * ic + 4
                exs, exs2 = carry.pop(ic)

                def trim_of(jc):
                    koff = jc - 4 * ic
                    return P * koff if koff > 0 else 0

                for h in range(HPC):
                    avt = av_p.tile([P, 512], F32, tag="av", name=f"av{ic}{h}")
                    for jc in range(n_j):
                        t = trim_of(jc)
                        if h < 2:
                            exap = exs[jc][:, h, t:]
                        else:
                            exap = exs2[jc // 2][:, jc % 2, t:]
                        nc.tensor.matmul(
                            avt[0 : HD + 1, t:],
                            lhsT=v1[:, jc, h, :],
                            rhs=exap,
                            start=(jc == 0),
                            stop=(jc == n_j - 1),
                        )
                    # normalize: zr = 1/Z (DVE, psum read), partition-broadcast
                    # via a DRAM round-trip DMA, ctx = av * zb (DVE, psum read)
                    zr = zr_p.tile([1, 512], F32, tag="zrow", name=f"zr{ic}{h}")
                    nc.vector.reciprocal(zr, avt[HD : HD + 1, :])
                    zd = zd_p.tile([1, 512], F32, tag="zd", name=f"zd{ic}{h}")
                    nc.sync.dma_start(zd, zr)
                    zb = zr_p.tile([HD, 512], F32, tag="zb", name=f"zb{ic}{h}")
                    nc.sync.dma_start(zb, zd[:].to_broadcast((HD, 512)))
                    nc.vector.tensor_tensor(
                        ctxT[:, h, ts(ic, 512)], avt[0:HD, :], zb, MUL
                    )

            # ---------------- main schedule ----------------
            emit_qkv_proj(0)
            emit_v_proj(0)
            for ic in range(IC):
                # (64,128) phase: scores(ic) + oproj(ic-1) filler
                emit_scores(ic)
                # (128,128) phase: QK proj(ic+1), AV+norm(ic), V proj(ic+1)
                if ic + 1 < IC:
                    emit_qkv_proj(ic + 1)
                emit_av(ic)
                if ic + 1 < IC:
                    emit_v_proj(ic + 1)
                emit_oproj(ic)
            drain_filler(len(pend))

    _split_excess_waits(nc)
    return nc


_NC = None


def _get_nc():
    global _NC
    if _NC is None:
        _NC = _build_module()
    return _NC


def _make_mask():
    p = np.arange(P)[:, None]
    f = np.arange(512)[None, :]
    m = np.empty((P, 4, 2, 512), np.float32)
    for k in range(4):
        m[:, k, 0, :] = (p <= f - P * k).astype(np.float32)
        m[:, k, 1, :] = m[:, k, 0, :]
    return m.astype(NPBF16)


def _build_in_maps(x, wq, bq, wk, bk, wv, bv, wo):
    scale = 1.0 / np.sqrt(HD)
    mask = _make_mask()
    in_maps = []
    for core in range(8):
        b = core // 4
        h0 = (core % 4) * HPC
        heads = list(range(h0, h0 + HPC))

        # pair-packed slices: [Q_h0|Q_h1], [K_h0|K_h1], [Q_h2|K_h2]
        wqk = np.empty((D, DQK), np.float32)
        bqk = np.empty((P, HPC), np.float32)
        cs = [slice((h0 + i) * HD, (h0 + i + 1) * HD) for i in range(HPC)]
        wqk[:, 0:HD] = wq[:, cs[0]] * scale
        wqk[:, HD:P] = wq[:, cs[1]] * scale
        wqk[:, P : P + HD] = wk[:, cs[0]]
        wqk[:, P + HD : 2 * P] = wk[:, cs[1]]
        wqk[:, 2 * P : 2 * P + HD] = wq[:, cs[2]] * scale
        wqk[:, 2 * P + HD : 3 * P] = wk[:, cs[2]]
        bqk[0:HD, 0] = bq[cs[0]] * scale
        bqk[HD:P, 0] = bq[cs[1]] * scale
        bqk[0:HD, 1] = bk[cs[0]]
        bqk[HD:P, 1] = bk[cs[1]]
        bqk[0:HD, 2] = bq[cs[2]] * scale
        bqk[HD:P, 2] = bk[cs[2]]

        vcols = slice(h0 * HD, (h0 + HPC) * HD)
        wos = (
            wo[vcols, :].reshape(HPC, HD, D).transpose(1, 0, 2)
        )  # [HD, HPC, D]

        in_maps.append(
            {
                "xt": np.ascontiguousarray(x[b].T).astype(NPBF16),
                "wqk": wqk.astype(NPBF16),
                "bqk": bqk.astype(np.float32),
                "wv": np.ascontiguousarray(wv[:, vcols]).astype(NPBF16),
                "wos": np.ascontiguousarray(wos).astype(NPBF16),
                "mask": mask,
            }
        )
    return in_maps


def kernel(x, wq, bq, wk, bk, wv, bv, wo, bo):
    x = np.asarray(x, np.float32)
    wq = np.asarray(wq, np.float32)
    bq = np.asarray(bq, np.float32)
    wk = np.asarray(wk, np.float32)
    bk = np.asarray(bk, np.float32)
    wv = np.asarray(wv, np.float32)
    bv = np.asarray(bv, np.float32)
    wo = np.asarray(wo, np.float32)
    bo = np.asarray(bo, np.float32)

    in_maps = _build_in_maps(x, wq, bq, wk, bk, wv, bv, wo)
    res = run_bass_kernel_spmd(_get_nc(), in_maps, core_ids=list(range(8)))
    out = np.zeros((B, S, D), np.float32)
    for core in range(8):
        out[core // 4] += np.asarray(res.results[core]["out"], np.float32)
    out += bo + bv @ wo
    return out


# revision 10
# speedup vs baseline: 1.1344x; 1.1344x over previous
# Multi-head causal self-attention (B=2, S=2048, D=768, H=12) on 8 NeuronCores.
#
# Sharding: (batch, head-group) across cores. Core c handles batch c//4 and
# heads 3*(c%4) .. 3*(c%4)+2. Each core computes its heads' Q/K/V projections
# (column-sharded), the causal attention for those heads, and a row-sharded
# partial of the output projection. Host sums the 4 partials per batch + bo.
#
# Engine plan (v4):
#  - PE batches work by tile-size mode so the array never mode-switch-drains
#    mid-stream: (128,128) for QK/V projections and AV (M=65, ones column
#    accumulates the softmax denominator); (64,128) for scores (heads 0,1
#    pair-stacked on partitions and issued to row tiles T0/T8 which run
#    concurrently; head 2 on T0) and the K=64 output projection.
#  - Attention chunks are processed in DESCENDING size order (ic3..ic0) with
#    all QK projections hoisted to the front: the long exp streams start
#    early and the kernel tail is the smallest chunk.
#  - ACT runs only exp. 1/Z = DVE reciprocal, partition-broadcast via a DRAM
#    round-trip DMA; the normalize multiplies are deferred in the DVE queue
#    so the DMA latency never blocks projection evacuations.
#  - GPSIMD: causal-mask multiplies. Output DMA'd in bf16; host sums in fp32.

import sys

import ml_dtypes
import numpy as np

sys.path.insert(0, "/opt/trn_rl_repo")

import concourse.bass as bass  # noqa: E402
import concourse.mybir as mybir  # noqa: E402
import concourse.tile as tile  # noqa: E402
from concourse.bass import ts  # noqa: E402
from concourse.bass_utils import run_bass_kernel_spmd  # noqa: E402

F32 = mybir.dt.float32
BF16 = mybir.dt.bfloat16
AF = mybir.ActivationFunctionType
MUL = mybir.AluOpType.mult
ADD = mybir.AluOpType.add
NPBF16 = ml_dtypes.bfloat16

B, S, D, H, HD = 2, 2048, 768, 12, 64
HPC = 3               # heads per core
DQK = 2 * HPC * HD    # 384
DV = HPC * HD         # 192
P = 128
IC = S // 512         # 4 query chunks of 512
KC = D // P           # 6 contraction chunks
NIO = S // P          # 16 token chunks of 128


def _split_excess_waits(nc, max_waits=1):
    # walrus in this env rejects instructions carrying more than ~1-2
    # sync-waits. Move excess waits onto preceding same-engine nops.
    n_split = 0
    for func in nc.m.functions:
        for blk in func.blocks:
            insts = blk.instructions
            out = []
            changed = False
            for inst in insts:
                si = inst.sync_info
                waits = list(si.on_wait) if si and si.on_wait else []
                if len(waits) > max_waits:
                    changed = True
                    for j, w in enumerate(waits[:-max_waits]):
                        out.append(
                            mybir.InstNoOp(
                                name=f"{inst.name}-wsplit{j}",
                                engine=inst.engine,
                                ins=[],
                                outs=[],
                                sync_info=mybir.SyncInfo(
                                    on_wait=[w], on_update=[]
                                ),
                            )
                        )
                        n_split += 1
                    inst.sync_info = mybir.SyncInfo(
                        on_wait=waits[-max_waits:],
                        on_update=list(si.on_update) if si.on_update else [],
                    )
                out.append(inst)
            if changed:
                blk.instructions = out
    return n_split


def _build_module():
    nc = bass.Bass()
    xt_d = nc.dram_tensor("xt", [D, S], BF16, kind="ExternalInput")
    wqk_d = nc.dram_tensor("wqk", [D, DQK], BF16, kind="ExternalInput")
    bqk_d = nc.dram_tensor("bqk", [P, HPC], F32, kind="ExternalInput")
    wv_d = nc.dram_tensor("wv", [D, DV], BF16, kind="ExternalInput")
    wos_d = nc.dram_tensor("wos", [HD, HPC, D], BF16, kind="ExternalInput")
    mask_d = nc.dram_tensor("mask", [P, 4, 2, 512], BF16, kind="ExternalInput")
    out_d = nc.dram_tensor("out", [S, D], BF16, kind="ExternalOutput")
    scratch_d = nc.dram_tensor("scratch", [HD + 1, 512], F32)

    with tile.TileContext(nc) as tc:
        with (
            tc.tile_pool(name="const", bufs=1) as cp,
            tc.tile_pool(name="exp", bufs=44) as exp_p,
            tc.tile_pool(name="zr", bufs=4) as zr_p,
            tc.tile_pool(name="zd", bufs=3, space="DRAM") as zd_p,
            tc.tile_pool(name="outp", bufs=2) as op,
            tc.tile_pool(name="proj", bufs=2, space="PSUM") as proj_p,
            tc.tile_pool(name="scps", bufs=2, space="PSUM") as sc_p,
            tc.tile_pool(name="avps", bufs=2, space="PSUM") as av_p,
        ):
            # ---- PE warm-up source via DVE memset (gpsimd starts slowly) ----
            warm_src = cp.tile([P, 520], BF16)
            nc.vector.memset(warm_src, 1.0)

            # ---- resident SBUF tensors; DMAs split for queue parallelism ----
            wqk_sb = cp.tile([P, KC, DQK], BF16)
            wqk_r = wqk_d.rearrange("(kc p) d -> p kc d", p=P)
            for kc in range(KC):
                nc.sync.dma_start(wqk_sb[:, kc, :], wqk_r[:, kc, :])
            bqk_sb = cp.tile([P, HPC], F32)
            nc.sync.dma_start(bqk_sb, bqk_d[:])

            xt_sb = cp.tile([P, KC, S], BF16)
            xt_r = xt_d.rearrange("(kc p) t -> p kc t", p=P)
            for kc in range(KC):
                nc.sync.dma_start(xt_sb[:, kc, 0:512], xt_r[:, kc, 0:512])
            for kc in range(KC):
                for ic in range(1, IC):
                    nc.sync.dma_start(
                        xt_sb[:, kc, ts(ic, 512)], xt_r[:, kc, ts(ic, 512)]
                    )

            wv_sb = cp.tile([P, KC, DV], BF16)
            wv_r = wv_d.rearrange("(kc p) d -> p kc d", p=P)
            for kc in range(KC):
                nc.sync.dma_start(wv_sb[:, kc, :], wv_r[:, kc, :])

            wos_sb = cp.tile([HD, HPC, D], BF16)
            for h in range(HPC):
                nc.sync.dma_start(wos_sb[:, h, :], wos_d[:, h, :])
            mask_sb = cp.tile([P, 4, 2, 512], BF16)
            for k in range(4):
                nc.sync.dma_start(mask_sb[:, k, :, :], mask_d[:, k, :, :])

            # V with a ones column (col HD) for the softmax denominator
            v1 = cp.tile([P, NIO, HPC, HD + 1], BF16)
            nc.gpsimd.memset(v1, 1.0)

            # pair-stacked Q^T/K^T for heads 0,1; separate tiles for head 2
            qTp = cp.tile([P, S], BF16)
            klp = cp.tile([P, S], BF16)
            qT2 = cp.tile([HD, S], BF16)
            kl2 = cp.tile([HD, S], BF16)
            ctxT = cp.tile([HD, HPC, S], BF16)

            # ---- PE warm-up: (128,128)-mode matmuls ----
            warm_ps = av_p.tile([P, 512], F32, tag="av", name="warm")
            for w in range(14):
                nc.tensor.matmul(
                    warm_ps[0 : HD + 1, :],
                    lhsT=warm_src[:, 0:65],
                    rhs=warm_src[:, 0:512],
                    start=True,
                    stop=(w == 13),
                )
            warm_sb = zr_p.tile([HD + 1, 512], F32, tag="warm", name="warmsb")
            nc.vector.tensor_copy(warm_sb, warm_ps[0 : HD + 1, :])
            nc.sync.dma_start(scratch_d[:], warm_sb)

            carry = {}   # ic -> (pair ex tiles, h2 ex tiles)

            def emit_qkv_proj(ic):
                isl = ts(ic, 512)
                # slices: 0 -> [Q_h0|Q_h1], 1 -> [K_h0|K_h1], 2 -> [Q_h2|K_h2]
                for sl in range(HPC):
                    ps = proj_p.tile([P, 512], F32, tag="proj")
                    for kc in range(KC):
                        nc.tensor.matmul(
                            ps,
                            lhsT=wqk_sb[:, kc, ts(sl, P)],
                            rhs=xt_sb[:, kc, isl],
                            start=(kc == 0),
                            stop=(kc == KC - 1),
                        )
                    if sl < 2:
                        dst = (qTp if sl == 0 else klp)[:, isl]
                        nc.vector.tensor_scalar(
                            dst, ps, bqk_sb[:, sl : sl + 1], None, ADD,
                        )
                    else:
                        nc.vector.tensor_scalar(
                            qT2[:, isl], ps[0:HD, :],
                            bqk_sb[0:HD, sl : sl + 1], None, ADD,
                        )
                        nc.vector.tensor_scalar(
                            kl2[:, isl], ps[HD:P, :],
                            bqk_sb[HD:P, sl : sl + 1], None, ADD,
                        )

            def trim_of(jc, ic):
                koff = jc - 4 * ic
                return P * koff if koff > 0 else 0

            def sc_group_pair(ic, jc):
                t = trim_of(jc, ic)
                koff = jc - 4 * ic
                sc = sc_p.tile([P, 2, 512], F32, tag="sc", name=f"sp{ic}_{jc}")
                for h in range(2):
                    hsl = ts(h, HD)
                    nc.tensor.matmul(
                        sc[:, h, t:],
                        lhsT=klp[hsl, ts(jc, P)],
                        rhs=qTp[hsl, ic * 512 + t : (ic + 1) * 512],
                        start=True,
                        stop=True,
                    )
                ex = exp_p.tile([P, 2, 512], BF16, tag="ex", name=f"xp{ic}_{jc}")
                nc.scalar.activation(ex[:, :, t:], sc[:, :, t:], AF.Exp)
                if koff >= 0:
                    nc.gpsimd.tensor_tensor(
                        ex[:, :, t:], ex[:, :, t:],
                        mask_sb[:, koff, :, t:], MUL,
                    )
                carry[ic][0].append(ex)

            def sc_group_h2(ic, jb):
                sc = sc_p.tile([P, 2, 512], F32, tag="sc", name=f"s2_{ic}_{jb}")
                for k in range(2):
                    jc = jb + k
                    t = trim_of(jc, ic)
                    nc.tensor.matmul(
                        sc[:, k, t:],
                        lhsT=kl2[:, ts(jc, P)],
                        rhs=qT2[:, ic * 512 + t : (ic + 1) * 512],
                        start=True,
                        stop=True,
                    )
                ex = exp_p.tile([P, 2, 512], BF16, tag="ex", name=f"x2_{ic}_{jb}")
                koff = jb - 4 * ic
                if koff >= 0 and trim_of(jb + 1, ic) > 0:
                    for k in range(2):
                        t = trim_of(jb + k, ic)
                        nc.scalar.activation(ex[:, k, t:], sc[:, k, t:], AF.Exp)
                        nc.gpsimd.tensor_tensor(
                            ex[:, k, t:], ex[:, k, t:],
                            mask_sb[:, koff + k, 0, t:], MUL,
                        )
                else:
                    nc.scalar.activation(ex, sc, AF.Exp)
                    if koff >= 0:
                        nc.gpsimd.tensor_tensor(
                            ex, ex, mask_sb[:, koff : koff + 2, 0, :], MUL,
                        )
                carry[ic][1].append(ex)

            # deferred normalize mult; created after fin(h) ran (needs zb)
            zbs = {}

            def make_fin(ic, h, state):
                def go():
                    avt = state[h]
                    zr = zr_p.tile([1, 512], F32, tag="zrow", name=f"zr{ic}{h}")
                    nc.vector.reciprocal(zr, avt[HD : HD + 1, :])
                    zd = zd_p.tile([1, 512], F32, tag="zd", name=f"zd{ic}{h}")
                    nc.sync.dma_start(zd, zr)
                    zb = zr_p.tile([HD, 512], F32, tag="zb", name=f"zb{ic}{h}")
                    nc.sync.dma_start(zb, zd[:].to_broadcast((HD, 512)))
                    zbs[(ic, h)] = zb
                return go

            def make_mult(ic, h, state):
                def go():
                    avt = state.pop(h)
                    nc.vector.tensor_tensor(
                        ctxT[:, h, ts(ic, 512)], avt[0:HD, :],
                        zbs.pop((ic, h)), MUL,
                    )
                return go

            def av_stream(ic):
                # mm batches + fin, with each head's mult deferred one head
                n_j = 4 * ic + 4
                state = {}
                work = []

                def mk_mm(h, j0, j1):
                    def go():
                        if h not in state:
                            state[h] = av_p.tile(
                                [P, 512], F32, tag="av", name=f"av{ic}{h}"
                            )
                        avt = state[h]
                        exs, exs2 = carry[ic]
                        for jc in range(j0, j1):
                            t = trim_of(jc, ic)
                            exap = (exs[jc][:, h, t:] if h < 2
                                    else exs2[jc // 2][:, jc % 2, t:])
                            nc.tensor.matmul(
                                avt[0 : HD + 1, t:],
                                lhsT=v1[:, jc, h, :],
                                rhs=exap,
                                start=(jc == 0),
                                stop=(jc == n_j - 1),
                            )
                    return go

                for h in range(HPC):
                    for j0 in range(0, n_j, 4):
                        work.append(mk_mm(h, j0, min(j0 + 4, n_j)))
                    work.append(make_fin(ic, h, state))
                    if h >= 1:
                        # mult for the previous head (zb DMA has had a full
                        # head's worth of AV matmuls to land)
                        work.append(make_mult(ic, h - 1, state))
                work.append(make_mult(ic, HPC - 1, state))
                return work

            def oproj_units(ic):
                units = []
                for io4 in range(4):
                    io = ic * 4 + io4

                    def unit(io=io):
                        o_sb = op.tile([P, D], BF16, tag="osb")
                        for ot, ow in ((0, 512), (1, 256)):
                            ps = proj_p.tile([P, 512], F32, tag="proj")
                            pso = ps[:, :ow]
                            for h in range(HPC):
                                nc.tensor.matmul(
                                    pso,
                                    lhsT=ctxT[:, h, ts(io, P)],
                                    rhs=wos_sb[:, h, ot * 512 : ot * 512 + ow],
                                    start=(h == 0),
                                    stop=(h == HPC - 1),
                                )
                            nc.vector.tensor_copy(
                                o_sb[:, ot * 512 : ot * 512 + ow], pso
                            )
                        nc.sync.dma_start(out_d[ts(io, P), :], o_sb)

                    units.append(unit)
                return units

            def v_units():
                units = []
                for io in range(NIO):
                    def unit(io=io):
                        ps = proj_p.tile([P, 512], F32, tag="proj")
                        psv = ps[:, :DV]
                        c0 = io * P
                        for kc in range(KC):
                            nc.tensor.matmul(
                                psv,
                                lhsT=xt_sb[:, kc, c0 : c0 + P],
                                rhs=wv_sb[:, kc, :],
                                start=(kc == 0),
                                stop=(kc == KC - 1),
                            )
                        nc.vector.tensor_copy(
                            v1[:, io, :, 0:HD],
                            psv.rearrange("p (h e) -> p h e", e=HD),
                        )
                    units.append(unit)
                return units

            def run_phase(ic, work128, work64=()):
                # Emit scores for chunk ic in jb-slots; after each slot emit a
                # proportional share of 64-mode fillers (oproj) and 128-mode
                # work (V proj or AV of the larger chunk).
                carry[ic] = ([], [])
                n_j = 4 * ic + 4
                work64 = list(work64)
                work128 = list(work128)
                nslots = n_j // 2
                d64 = d128 = 0
                for s, jb in enumerate(range(0, n_j, 2)):
                    sc_group_pair(ic, jb)
                    sc_group_pair(ic, jb + 1)
                    sc_group_h2(ic, jb)
                    w = len(work64) * (s + 1) // nslots
                    while d64 < w:
                        work64[d64]()
                        d64 += 1
                    w = len(work128) * (s + 1) // nslots
                    while d128 < w:
                        work128[d128]()
                        d128 += 1

            # ---------------- main schedule ----------------
            for ic in range(IC):
                emit_qkv_proj(ic)
            run_phase(3, v_units())
            run_phase(2, av_stream(3))
            run_phase(1, av_stream(2), oproj_units(3))
            run_phase(0, av_stream(1), oproj_units(2))
            # tail: AV of chunk 0 interleaved with the last output projections
            tail128 = av_stream(0)
            tail64 = oproj_units(1) + oproj_units(0)
            while tail128 or tail64:
                for _ in range(2):
                    if tail128:
                        tail128.pop(0)()
                for _ in range(1):
                    if tail64:
                        tail64.pop(0)()

    _split_excess_waits(nc)
    return nc


_NC = None


def _get_nc():
    global _NC
    if _NC is None:
        _NC = _build_module()
    return _NC


def _make_mask():
    p = np.arange(P)[:, None]
    f = np.arange(512)[None, :]
    m = np.empty((P, 4, 2, 512), np.float32)
    for k in range(4):
        m[:, k, 0, :] = (p <= f - P * k).astype(np.float32)
        m[:, k, 1, :] = m[:, k, 0, :]
    return m.astype(NPBF16)


def _build_in_maps(x, wq, bq, wk, bk, wv, bv, wo):
    scale = 1.0 / np.sqrt(HD)
    mask = _make_mask()
    in_maps = []
    for core in range(8):
        b = core // 4
        h0 = (core % 4) * HPC

        # pair-packed slices: [Q_h0|Q_h1], [K_h0|K_h1], [Q_h2|K_h2]
        wqk = np.empty((D, DQK), np.float32)
        bqk = np.empty((P, HPC), np.float32)
        cs = [slice((h0 + i) * HD, (h0 + i + 1) * HD) for i in range(HPC)]
        wqk[:, 0:HD] = wq[:, cs[0]] * scale
        wqk[:, HD:P] = wq[:, cs[1]] * scale
        wqk[:, P : P + HD] = wk[:, cs[0]]
        wqk[:, P + HD : 2 * P] = wk[:, cs[1]]
        wqk[:, 2 * P : 2 * P + HD] = wq[:, cs[2]] * scale
        wqk[:, 2 * P + HD : 3 * P] = wk[:, cs[2]]
        bqk[0:HD, 0] = bq[cs[0]] * scale
        bqk[HD:P, 0] = bq[cs[1]] * scale
        bqk[0:HD, 1] = bk[cs[0]]
        bqk[HD:P, 1] = bk[cs[1]]
        bqk[0:HD, 2] = bq[cs[2]] * scale
        bqk[HD:P, 2] = bk[cs[2]]

        vcols = slice(h0 * HD, (h0 + HPC) * HD)
        wos = (
            wo[vcols, :].reshape(HPC, HD, D).transpose(1, 0, 2)
        )  # [HD, HPC, D]

        in_maps.append(
            {
                "xt": np.ascontiguousarray(x[b].T).astype(NPBF16),
                "wqk": wqk.astype(NPBF16),
                "bqk": bqk.astype(np.float32),
                "wv": np.ascontiguousarray(wv[:, vcols]).astype(NPBF16),
                "wos": np.ascontiguousarray(wos).astype(NPBF16),
                "mask": mask,
            }
        )
    return in_maps


def kernel(x, wq, bq, wk, bk, wv, bv, wo, bo):
    x = np.asarray(x, np.float32)
    wq = np.asarray(wq, np.float32)
    bq = np.asarray(bq, np.float32)
    wk = np.asarray(wk, np.float32)
    bk = np.asarray(bk, np.float32)
    wv = np.asarray(wv, np.float32)
    bv = np.asarray(bv, np.float32)
    wo = np.asarray(wo, np.float32)
    bo = np.asarray(bo, np.float32)

    in_maps = _build_in_maps(x, wq, bq, wk, bk, wv, bv, wo)
    res = run_bass_kernel_spmd(_get_nc(), in_maps, core_ids=list(range(8)))
    out = np.zeros((B, S, D), np.float32)
    for core in range(8):
        out[core // 4] += np.asarray(res.results[core]["out"], np.float32)
    out += bo + bv @ wo
    return out
